# revision 11
# baseline (speedup 1.0000x reference)
"""Trainium2 Bass kernel for LlamaAttention (B=2, S=2048, H=2048, NH=32, KVH=8, HD=64).

Sharding: tensor-parallel over heads across 8 cores (1 KV head + 4 Q heads per
core), flash-style causal attention, then AllToAll to row-shard o_proj so each
core emits a disjoint [512, 2048] slice of the output; host concatenates.
"""
import sys
import numpy as np

sys.path.insert(0, "/opt/trn_rl_repo")

import concourse.bass as bass
import concourse.mybir as mybir
import concourse.tile as tile
from concourse import bacc
from concourse import bass_utils

B, S, H = 2, 2048, 2048
NH, KVH, HD = 32, 8, 64
G = NH // KVH          # 4 q-heads per kv head
N_CORES = 8
EQ = G * HD            # 256 q-cols per core
SCALE = 1.0 / float(np.sqrt(HD))
BASE = 10000.0
BS = B * S             # 4096
ROWS = BS // N_CORES   # 512 output rows per core
NEG = -280.0           # additive pre-scale mask constant: SCALE*NEG = -35

F32 = mybir.dt.float32
Exp = mybir.ActivationFunctionType.Exp

_cache = {}


def _build(variant):
    """variant: 'causal' (skip masked blocks) or 'dense' (all blocks, optional
    additive mask input 'maskT' [B, S(j), S(i)])."""
    nc = bacc.Bacc("TRN2", target_bir_lowering=False, debug=False,
                   num_devices=N_CORES)
    hidT = nc.dram_tensor("hidT", [H, BS], F32, kind="ExternalInput").ap()
    wqkv = nc.dram_tensor("wqkv", [H, EQ + 2 * HD], F32, kind="ExternalInput").ap()
    wo = nc.dram_tensor("wo", [H, H], F32, kind="ExternalInput").ap()
    cos4 = nc.dram_tensor("cos4", [128, BS], F32, kind="ExternalInput").ap()
    sin4 = nc.dram_tensor("sin4", [128, BS], F32, kind="ExternalInput").ap()
    maskc = nc.dram_tensor("maskc", [128, 896], F32, kind="ExternalInput").ap()
    rot2 = nc.dram_tensor("rot2", [128, 64], F32, kind="ExternalInput").ap()
    use_mask = variant == "dense_mask"
    if use_mask:
        maskT = nc.dram_tensor("maskT", [B, S, S], F32, kind="ExternalInput").ap()
    out = nc.dram_tensor("out", [ROWS, H], F32, kind="ExternalOutput").ap()

    cc_in = nc.dram_tensor("cc_in", [N_CORES, EQ, ROWS], F32, kind="Internal").ap()
    cc_out = nc.dram_tensor("cc_out", [N_CORES, EQ, ROWS], F32, kind="Internal").ap()

    causal = variant == "causal"
    NB = S // 512          # 4 i-blocks per batch
    NJ = S // 128          # 16 j-tiles per batch

    with tile.TileContext(nc) as tc:
        with tc.tile_pool(name="persist", bufs=1) as persist:
            # constants
            ident = persist.tile([128, 128], F32)
            from concourse.masks import make_identity
            make_identity(nc, ident)
            ones65 = persist.tile([65, 64], F32)
            nc.vector.memset(ones65, 1.0)
            rot_sb = persist.tile([128, 64], F32)
            nc.sync.dma_start(out=rot_sb, in_=rot2)
            maskc_sb = persist.tile([128, 896], F32)
            nc.sync.dma_start(out=maskc_sb, in_=maskc)
            cos_sb = persist.tile([128, BS], F32)
            sin_sb = persist.tile([128, BS], F32)
            nc.sync.dma_start(out=cos_sb, in_=cos4)
            nc.sync.dma_start(out=sin_sb, in_=sin4)

            qT = persist.tile([128, 2, BS], F32)     # 4 q-heads: [64*(h%2), h//2]
            kT2 = persist.tile([128, BS], F32)       # kT duplicated on both bases
            vp = persist.tile([128, 2 * NJ, HD + 1], F32)  # V' per (b, jt), ones col

            # ---------------- Phase 1: fused QKV projection + RoPE ----------
            # RoPE: rot = rotate_half(x) computed on PE via signed permutation
            # matrix (partition move); then x*cos + rot*sin on DVE, all
            # partition-aligned.
            with tc.tile_pool(name="vtmp", bufs=2) as vtp:
              vT_sb = vtp.tile([64, BS], F32)
              with tc.tile_pool(name="wq", bufs=1) as wqp, \
                   tc.tile_pool(name="hid", bufs=3) as hp, \
                   tc.tile_pool(name="raw", bufs=2) as rawp, \
                   tc.tile_pool(name="qkps", bufs=1, space="PSUM") as qkps, \
                   tc.tile_pool(name="rops", bufs=1, space="PSUM") as rops:
                wq_sb = wqp.tile([128, 16, EQ + 2 * HD], F32)
                nc.sync.dma_start(
                    out=wq_sb, in_=wqkv.rearrange("(ho p) e -> p ho e", p=128))
                for sc in range(4):          # s-chunks of 1024
                    c0 = sc * 1024
                    ps = [qkps.tile([128, 1024], F32, tag=f"pe{e}",
                                    name=f"ps_{sc}_{e}")
                          for e in range(3)]
                    for ht in range(16):
                        ht_sb = hp.tile([128, 1024], F32)
                        nc.sync.dma_start(
                            out=ht_sb, in_=hidT[ht * 128:(ht + 1) * 128,
                                               c0:c0 + 1024])
                        for e in range(3):
                            for sb in range(2):
                                nc.tensor.matmul(
                                    ps[e][:, sb * 512:(sb + 1) * 512],
                                    wq_sb[:, ht, e * 128:(e + 1) * 128],
                                    ht_sb[:, sb * 512:(sb + 1) * 512],
                                    start=(ht == 0), stop=(ht == 15))
                    cs = cos_sb[:, c0:c0 + 1024]
                    sn = sin_sb[:, c0:c0 + 1024]
                    tmp = rawp.tile([128, 1024], F32, tag="ropetmp")
                    for e in range(3):
                        # copy psum -> sbuf raw (also the non-rope path for v)
                        raw = rawp.tile([128, 1024], F32, tag=f"raw{e}")
                        nc.vector.tensor_copy(raw, ps[e])
                        nrows = 128 if e < 2 else 64
                        rot_ps = rops.tile([128, 1024], F32, tag="rot")
                        for sb in range(2):
                            nc.tensor.matmul(
                                rot_ps[0:64, sb * 512:(sb + 1) * 512],
                                rot_sb[0:64, :],
                                raw[0:64, sb * 512:(sb + 1) * 512],
                                start=True, stop=True, tile_position=(0, 0))
                            if nrows == 128:
                                nc.tensor.matmul(
                                    rot_ps[64:128, sb * 512:(sb + 1) * 512],
                                    rot_sb[64:128, :],
                                    raw[64:128, sb * 512:(sb + 1) * 512],
                                    start=True, stop=True, tile_position=(64, 64))
                        dst = (qT[:, e, c0:c0 + 1024] if e < 2
                               else kT2[0:64, c0:c0 + 1024])
                        r = slice(0, nrows)
                        nc.vector.tensor_mul(tmp[r], rot_ps[r], sn[r])
                        nc.vector.tensor_mul(dst, raw[r], cs[r])
                        nc.vector.tensor_add(dst, dst, tmp[r])
                        if e == 2:
                            # duplicate kT to partitions 64:128 and shift v
                            # rows 64:128 -> vT_sb rows 0:64 (DMA partition move)
                            nc.sync.dma_start(out=kT2[64:128, c0:c0 + 1024],
                                              in_=kT2[0:64, c0:c0 + 1024])
                            nc.sync.dma_start(out=vT_sb[:, c0:c0 + 1024],
                                              in_=raw[64:128, :])

              # ---- build V' via PE transpose of vT ----
              nc.vector.memset(vp[:, :, HD:HD + 1], 1.0)
              with tc.tile_pool(name="tps", bufs=4, space="PSUM") as tps:
                  for t in range(2 * NJ):
                      pt = tps.tile([128, 64], F32, name=f"pt{t % 4}", tag="pt")
                      nc.tensor.transpose(
                          pt, vT_sb[:, t * 128:(t + 1) * 128], ident[0:64, 0:64])
                      nc.vector.tensor_copy(vp[:, t, 0:HD], pt)

            # ---------------- Phase 2: flash attention per (b, h) -----------
            with tc.tile_pool(name="probs", bufs=4) as pp, \
                 tc.tile_pool(name="ctxs", bufs=4) as cs_pool, \
                 tc.tile_pool(name="sps", bufs=4, space="PSUM") as sps, \
                 tc.tile_pool(name="cps", bufs=2, space="PSUM") as cps, \
                 tc.tile_pool(name="rps", bufs=2, space="PSUM") as rps:
                for b in range(B):
                    for h in range(G):
                        hb = 64 * (h % 2)
                        tp = (hb, 0)
                        for ib in range(NB):
                            i0 = b * S + ib * 512
                            ctx_ps = cps.tile([65, 512], F32, tag="ctx")
                            jts = range(4 * (ib + 1)) if causal else range(NJ)
                            njt = len(jts)
                            for jt in jts:
                                j0 = b * S + jt * 128
                                s_ps = sps.tile([128, 512], F32, tag="s")
                                nc.tensor.matmul(
                                    s_ps,
                                    kT2[hb:hb + 64, j0:j0 + 128],
                                    qT[hb:hb + 64, h // 2, i0:i0 + 512],
                                    start=True, stop=True, tile_position=tp)
                                if causal and jt >= 4 * ib:
                                    d = 384 - 128 * (jt - 4 * ib)
                                    nc.vector.tensor_add(
                                        s_ps, s_ps, maskc_sb[:, d:d + 512])
                                if use_mask:
                                    m_sb = pp.tile([128, 512], F32, tag="mt")
                                    nc.sync.dma_start(
                                        out=m_sb,
                                        in_=maskT[b, jt * 128:(jt + 1) * 128,
                                                  ib * 512:(ib + 1) * 512])
                                    nc.vector.tensor_add(s_ps, s_ps, m_sb)
                                pr = pp.tile([128, 512], F32, tag="pr")
                                nc.scalar.activation(out=pr, in_=s_ps, func=Exp,
                                                     scale=SCALE)
                                nc.tensor.matmul(
                                    ctx_ps, vp[:, b * NJ + jt, :], pr,
                                    start=(jt == jts[0]), stop=(jt == njt - 1 + jts[0]))
                            # normalize by row-sums (psum row 64)
                            rec = cs_pool.tile([65, 512], F32, tag="rec")
                            nc.vector.reciprocal(rec[64:65, :], ctx_ps[64:65, :])
                            rb_ps = rps.tile([64, 512], F32, tag="rb")
                            nc.tensor.matmul(rb_ps, ones65[64:65, :],
                                             rec[64:65, :], start=True, stop=True,
                                             tile_position=(64, 0))
                            rb = cs_pool.tile([64, 512], F32, tag="rbs")
                            nc.vector.tensor_copy(rb, rb_ps)
                            ctxn = cs_pool.tile([64, 512], F32, tag="ctxn")
                            nc.vector.tensor_mul(ctxn, ctx_ps[0:64, :], rb)
                            nc.sync.dma_start(
                                out=cc_in[4 * b + ib, h * 64:(h + 1) * 64, :],
                                in_=ctxn)

            # ---------------- Phase 3: AllToAll ------------------------------
            nc.gpsimd.collective_compute(
                "AllToAll", mybir.AluOpType.bypass,
                replica_groups=[list(range(N_CORES))],
                ins=[cc_in], outs=[cc_out])

        # ---------------- Phase 4: o_proj (row-sharded) ----------------------
        ccf = cc_out.rearrange("r e i -> (r e) i").rearrange(
            "(et p) i -> p et i", p=128)
        with tc.tile_pool(name="ctxf", bufs=1) as cfp, \
             tc.tile_pool(name="wop", bufs=2) as wop, \
             tc.tile_pool(name="osb", bufs=4) as osb, \
             tc.tile_pool(name="ops", bufs=4, space="PSUM") as ops:
            ctxF = cfp.tile([128, 16, ROWS], F32)
            nc.sync.dma_start(out=ctxF, in_=ccf)
            for ob in range(4):
                wo_sb = wop.tile([128, 16, 512], F32)
                nc.sync.dma_start(
                    out=wo_sb,
                    in_=wo.rearrange("(et p) o -> p et o", p=128)[
                        :, :, ob * 512:(ob + 1) * 512])
                for it in range(4):
                    o_ps = ops.tile([128, 512], F32, tag="o")
                    for et in range(16):
                        nc.tensor.matmul(
                            o_ps, ctxF[:, et, it * 128:(it + 1) * 128],
                            wo_sb[:, et, :], start=(et == 0), stop=(et == 15))
                    o_sb = osb.tile([128, 512], F32, tag="os")
                    nc.scalar.copy(o_sb, o_ps)
                    nc.sync.dma_start(
                        out=out[it * 128:(it + 1) * 128, ob * 512:(ob + 1) * 512],
                        in_=o_sb)
    nc.compile()
    return nc


def _rope_tables(position_ids):
    inv_freq = 1.0 / (BASE ** (np.arange(0, HD, 2, dtype=np.float32) / HD))
    t = np.arange(S, dtype=np.float32)
    freqs = np.outer(t, inv_freq)                     # [S, 32]
    cos_t, sin_t = np.cos(freqs), np.sin(freqs)
    pos = np.asarray(position_ids).reshape(-1)        # [BS]
    cos = cos_t[pos].T.astype(np.float32)             # [32, BS]
    sin = sin_t[pos].T.astype(np.float32)
    return (np.tile(cos, (4, 1)).copy(), np.tile(sin, (4, 1)).copy())


def _rot_mat():
    # R[p, m] per 64-block: out[m] = sum_p R[p, m] x[p] = rotate_half(x)[m]
    R = np.zeros((64, 64), np.float32)
    for m in range(32):
        R[m + 32, m] = -1.0
    for m in range(32, 64):
        R[m - 32, m] = 1.0
    return np.concatenate([R, R], axis=0).copy()  # [128, 64]


def _mask_const():
    jj = np.arange(128)[:, None]
    u = np.arange(896)[None, :]
    return np.where(jj <= u - 384, 0.0, NEG).astype(np.float32)


def _detect_variant(attention_mask):
    am = np.asarray(attention_mask)
    if not am.any():
        return "dense_nomask"
    causal = np.where(np.tril(np.ones((S, S), dtype=bool)), 0.0, -1e9)
    causal = np.broadcast_to(causal.astype(np.float32)[None, None], (B, 1, S, S))
    if np.array_equal(am, causal):
        return "causal"
    return "dense_mask"


def kernel(hidden_states, attention_mask, position_ids, Wq, Wk, Wv, Wo):
    variant = _detect_variant(attention_mask)
    if variant not in _cache:
        _cache[variant] = _build(variant)
    nc = _cache[variant]

    hidT = np.ascontiguousarray(
        np.asarray(hidden_states, dtype=np.float32).reshape(BS, H).T)
    cos4, sin4 = _rope_tables(position_ids)
    maskc = _mask_const()
    Wo_f = np.ascontiguousarray(np.asarray(Wo, dtype=np.float32))
    in_maps = []
    for c in range(N_CORES):
        wqkv = np.ascontiguousarray(np.concatenate([
            Wq[:, c * EQ:(c + 1) * EQ],
            Wk[:, c * HD:(c + 1) * HD],
            Wv[:, c * HD:(c + 1) * HD]], axis=1).astype(np.float32))
        m = {"hidT": hidT, "wqkv": wqkv, "wo": Wo_f,
             "cos4": cos4, "sin4": sin4, "maskc": maskc, "rot2": _rot_mat()}
        if variant == "dense_mask":
            m["maskT"] = np.ascontiguousarray(
                np.asarray(attention_mask, dtype=np.float32)[:, 0]
                .transpose(0, 2, 1))
        in_maps.append(m)

    res = bass_utils.run_bass_kernel_spmd(nc, in_maps,
                                          core_ids=list(range(N_CORES)))
    out = np.concatenate([res.results[c]["out"] for c in range(N_CORES)], axis=0)
    return out.reshape(B, S, H)


if __name__ == "__main__":
    rng = np.random.default_rng(0)
    hs = rng.standard_normal((B, S, H), dtype=np.float32)
    causal = np.where(np.tril(np.ones((S, S), dtype=bool)), 0.0, -1e9)
    am = np.broadcast_to(causal.astype(np.float32)[None, None], (B, 1, S, S))
    pid = np.broadcast_to(np.arange(S, dtype=np.int32), (B, S))
    w = 0.02
    Wq = (rng.standard_normal((H, NH * HD), dtype=np.float32) * w)
    Wk = (rng.standard_normal((H, KVH * HD), dtype=np.float32) * w)
    Wv = (rng.standard_normal((H, KVH * HD), dtype=np.float32) * w)
    Wo = (rng.standard_normal((NH * HD, H), dtype=np.float32) * w)
    o = kernel(hidden_states=hs, attention_mask=am, position_ids=pid,
               Wq=Wq, Wk=Wk, Wv=Wv, Wo=Wo)
    print("out", o.shape, o.dtype, float(np.abs(o).mean()))
